# revision 1
# baseline (speedup 1.0000x reference)
"""Bidirectional LSTM kernel for 8 Trainium2 NeuronCores.

Strategy: data-parallel over (batch, direction). Cores 0-3 run the forward
LSTM on 8 examples each; cores 4-7 run the backward LSTM (on host-flipped
sequences) on 8 examples each. All cores run one SPMD program; only the
input data (weights / sequences) differs per core.

Device layout is fully transposed: gate pre-activations z.T live as
[2048 gate-units (16 chunks of 128 partitions), 8 batch] so the per-step
elementwise work uses all 128 partitions. The input projection x@Wi+b is a
single big bf16 GEMM whose (transposed) result stays SBUF-resident; the
recurrence then runs 512 steps of h@Wh with Wh tiles stationary in the PE
array and h.T [128, 8] moving, fp32 PSUM accumulation, fp32 cell state.
"""

import sys

if "/opt/trn_rl_repo" not in sys.path:
    sys.path.insert(0, "/opt/trn_rl_repo")

import ml_dtypes
import numpy as np

B, T, D, HS = 32, 512, 1024, 1024
H = HS // 2          # 512 per-direction hidden
G = 4 * H            # 2048 gate width
BC = 8               # batch per core
NCORES = 8
KD = D // 128        # 8 contraction chunks for x@Wi
KH = H // 128        # 4 contraction chunks for h@Wh
M = G // 128         # 16 gate-unit chunks
CB = 512             # proj column-block (t,b columns per psum tile)
NCB = (T * BC) // CB # 8 column blocks
FLUSH = 8            # steps between output DMA flushes

BF16 = ml_dtypes.bfloat16

_CACHE = {}


def _build_program(n_steps=T):
    import concourse.mybir as mybir
    import concourse.tile as tile
    from concourse import bacc

    f32 = mybir.dt.float32
    bf = mybir.dt.bfloat16
    Sig = mybir.ActivationFunctionType.Sigmoid
    Tanh = mybir.ActivationFunctionType.Tanh

    nc = bacc.Bacc("TRN2", target_bir_lowering=False, debug=False)
    xT = nc.dram_tensor("xT", [D, T * BC], bf, kind="ExternalInput").ap()
    Wi = nc.dram_tensor("Wi", [D, G], bf, kind="ExternalInput").ap()
    Wh = nc.dram_tensor("Wh", [H, G], bf, kind="ExternalInput").ap()
    bias = nc.dram_tensor("bias", [128, M], f32, kind="ExternalInput").ap()
    hout = nc.dram_tensor("hout", [n_steps, KH, 128, BC], f32, kind="ExternalOutput").ap()

    with tile.TileContext(nc) as tc:
        with (
            tc.tile_pool(name="const", bufs=1) as const_pool,
            tc.tile_pool(name="xtcb", bufs=2) as xt_pool,
            tc.tile_pool(name="wim", bufs=3) as wi_pool,
            tc.tile_pool(name="work", bufs=3) as work_pool,
            tc.tile_pool(name="outstage", bufs=2) as out_pool,
            tc.tile_pool(name="psum_proj", bufs=4, space="PSUM") as psum_proj,
            tc.tile_pool(name="psum_rec", bufs=4, space="PSUM") as psum_rec,
        ):
            # ---- constants / persistent state ----
            wh_sb = const_pool.tile([128, KH, G], bf)
            nc.sync.dma_start(wh_sb, Wh.rearrange("(k p) g -> p k g", p=128))
            bias_sb = const_pool.tile([128, M], f32)
            nc.sync.dma_start(bias_sb, bias)
            xproj = const_pool.tile([128, M, T * BC], bf)  # 128KB/partition
            h_bf = const_pool.tile([128, KH, BC], bf)
            c_st = const_pool.tile([128, KH, BC], f32)
            nc.any.memset(h_bf, 0.0)
            nc.any.memset(c_st, 0.0)

            # ---- phase 1: xproj[:, m, t*8+b] = (x @ Wi + b).T  (bf16) ----
            for cb in range(NCB):
                xt_cb = xt_pool.tile([128, KD, CB], bf)
                nc.sync.dma_start(
                    xt_cb,
                    xT[:, cb * CB:(cb + 1) * CB].rearrange("(k p) c -> p k c", p=128),
                )
                for m in range(M):
                    wi_m = wi_pool.tile([128, KD, 128], bf)
                    nc.sync.dma_start(
                        wi_m,
                        Wi[:, m * 128:(m + 1) * 128].rearrange(
                            "(k p) g -> p k g", p=128
                        ),
                    )
                    ps = psum_proj.tile([128, CB], f32)
                    for k in range(KD):
                        nc.tensor.matmul(
                            ps,
                            wi_m[:, k, :],
                            xt_cb[:, k, :],
                            start=(k == 0),
                            stop=(k == KD - 1),
                        )
                    # evict + bias (per-partition scalar), downcast to bf16
                    nc.vector.tensor_scalar_add(
                        xproj[:, m, cb * CB:(cb + 1) * CB], ps, bias_sb[:, m:m + 1]
                    )

            # ---- phase 2: recurrence ----
            out_stage = None
            for t in range(n_steps):
                zt = psum_rec.tile([128, M, BC], f32)
                for m in range(M):
                    for k in range(KH):
                        # accumulation-group matmuls must stay consecutive:
                        # interleaving groups loses the start=True partials
                        nc.tensor.matmul(
                            zt[:, m, :],
                            wh_sb[:, k, m * 128:(m + 1) * 128],
                            h_bf[:, k, :],
                            start=(k == 0),
                            stop=(k == KH - 1),
                        )
                zf = work_pool.tile([128, M, BC], f32)
                nc.vector.tensor_add(zf, zt, xproj[:, :, t * BC:(t + 1) * BC])
                ac = work_pool.tile([128, M, BC], f32)
                nc.scalar.activation(ac[:, 0:8], zf[:, 0:8], Sig)     # i, f
                nc.scalar.activation(ac[:, 8:12], zf[:, 8:12], Tanh)  # g
                nc.scalar.activation(ac[:, 12:16], zf[:, 12:16], Sig) # o
                ig = work_pool.tile([128, KH, BC], f32)
                nc.vector.tensor_mul(ig, ac[:, 0:4], ac[:, 8:12])     # i*g
                nc.vector.tensor_mul(c_st, ac[:, 4:8], c_st)          # f*c
                nc.vector.tensor_add(c_st, c_st, ig)
                tc_t = work_pool.tile([128, KH, BC], f32)
                nc.scalar.activation(tc_t, c_st, Tanh)
                if t % FLUSH == 0:
                    out_stage = out_pool.tile([128, FLUSH, KH, BC], f32)
                hs = out_stage[:, t % FLUSH]
                nc.vector.tensor_mul(hs, ac[:, 12:16], tc_t)          # h = o*tanh(c)
                nc.vector.tensor_copy(h_bf, hs)                       # downcast bf16
                if t % FLUSH == FLUSH - 1:
                    nc.sync.dma_start(
                        hout[t - FLUSH + 1:t + 1].rearrange("f j p b -> p f j b"),
                        out_stage,
                    )
    nc.compile()
    return nc


def _get_program():
    if "nc" not in _CACHE:
        _CACHE["nc"] = _build_program(T)
    return _CACHE["nc"]


def _flip_sequences(x, lengths):
    """np equivalent of the reference's jax flip_sequences."""
    out = np.empty_like(x)
    for i in range(x.shape[0]):
        out[i] = np.flip(np.roll(x[i], x.shape[1] - int(lengths[i]), axis=0), axis=0)
    return out


def _core_inputs(x8, Wi, Wh, b):
    """x8: [8, T, D] fp32 -> per-core input dict."""
    xT = np.ascontiguousarray(x8.transpose(2, 1, 0).reshape(D, T * BC)).astype(BF16)
    return {
        "xT": xT,
        "Wi": Wi.astype(BF16),
        "Wh": Wh.astype(BF16),
        "bias": np.ascontiguousarray(b.reshape(M, 128).T).astype(np.float32),
    }


def kernel(inputs, lengths, Wi_f, Wh_f, b_f, Wi_b, Wh_b, b_b):
    from concourse.bass_utils import run_bass_kernel_spmd

    inputs = np.asarray(inputs, np.float32)
    lengths = np.asarray(lengths, np.int32)
    nc = _get_program()

    flipped = _flip_sequences(inputs, lengths)
    in_maps = []
    for c in range(4):
        in_maps.append(_core_inputs(inputs[c * BC:(c + 1) * BC], Wi_f, Wh_f, b_f))
    for c in range(4):
        in_maps.append(_core_inputs(flipped[c * BC:(c + 1) * BC], Wi_b, Wh_b, b_b))

    res = run_bass_kernel_spmd(nc, in_maps, list(range(NCORES)))

    fwd = np.empty((B, T, H), np.float32)
    bwd_flip = np.empty((B, T, H), np.float32)
    for c in range(4):
        # hout [T, KH, 128, BC] -> [BC, T, H]
        o = res.results[c]["hout"].transpose(3, 0, 1, 2).reshape(BC, T, H)
        fwd[c * BC:(c + 1) * BC] = o
        o = res.results[4 + c]["hout"].transpose(3, 0, 1, 2).reshape(BC, T, H)
        bwd_flip[c * BC:(c + 1) * BC] = o

    idx = np.maximum(0, lengths - 1)
    forward_final = fwd[np.arange(B), idx, :]
    backward_final = bwd_flip[np.arange(B), idx, :]
    bwd = _flip_sequences(bwd_flip, lengths)
    outputs = np.concatenate([fwd, bwd], axis=-1)
    return outputs, forward_final, backward_final
